# revision 8
# baseline (speedup 1.0000x reference)
"""Compositional attention kernel for Trainium2, 8-core SPMD.

Sharding: core c handles batch b = c // 4 and search-heads {2*(c%4), 2*(c%4)+1}
end-to-end (tensor-parallel over the S=8 search heads x data-parallel over
B=2).  Each core computes a partial y = out_heads @ w_out[head_rows] for its
batch in fp16; the host sums the 4 partials per batch in fp32.

All matmuls run in fp16 with fp32 PSUM accumulation.  The emission order keeps
the PE dense: each head's jt-loop emits scores -> exp -> first AV half
(i 0:1024, 2-tile lag) while the previous head's trailing work is drip-fed
into the same slots.  Trailing work is pipelined per 512-wide i-quarter:
AV second half, in-place Z tree (16->4) + ones-matmuls, un transposes, rqw,
and the stage-2 gating (exp-based, keeps the ACT table on Exp).  The
out-projection concatenates both heads (K=128, 16 matmuls).

SBUF aliasing: the last quarter of head-1's exp matrix reuses xT's buffer;
uf2 reuses sqT's buffer.
"""

import sys

import numpy as np

for _p in ("/opt/trn_rl_repo", "/root/.axon_site/_ro/trn_rl_repo"):
    if _p not in sys.path:
        sys.path.append(_p)

import concourse.bass as bass  # noqa: F401
import concourse.mybir as mybir
import concourse.tile as tile
from concourse import bacc
from concourse.bass_utils import run_bass_kernel_spmd
from concourse.masks import make_identity

S, R, DH = 8, 2, 64
B, N, DIM = 2, 2048, 512
P = 128
NT = N // P  # 16 j-tiles
KC = DIM // P  # 4 contraction chunks of x
NCORES = 8
HPC = 2  # heads per core

F32 = mybir.dt.float32
F16 = mybir.dt.float16
AF = mybir.ActivationFunctionType
ALU = mybir.AluOpType


def _emit(tc, xt, wq, wk, wv, wqr, wkt, wo, y, zscr):
    from contextlib import ExitStack

    nc = tc.nc
    with ExitStack() as ctx:
        cpool = ctx.enter_context(tc.tile_pool(name="const", bufs=1))
        xp = ctx.enter_context(tc.tile_pool(name="xp", bufs=1))
        qp = ctx.enter_context(tc.tile_pool(name="qp", bufs=1))
        sb = ctx.enter_context(tc.tile_pool(name="sb", bufs=1))
        hd = ctx.enter_context(tc.tile_pool(name="hd", bufs=2))
        # PSUM: st 2x[128,1024]f32 = 4 banks, av 1x[128,1024]f32 = 2 banks,
        # wk 2x[<=2KB] = 2 banks  -> 8 banks exactly.
        pst = ctx.enter_context(tc.tile_pool(name="pst", bufs=2, space="PSUM"))
        pav = ctx.enter_context(tc.tile_pool(name="pav", bufs=1, space="PSUM"))
        pwk = ctx.enter_context(tc.tile_pool(name="pwk", bufs=2, space="PSUM"))

        # ---- constants ----
        ident = cpool.tile([P, P], F16, name="ident")
        make_identity(nc, ident)
        ones16 = cpool.tile([P, 1], F16, name="ones16")
        nc.gpsimd.memset(ones16, 1.0)

        wq_sb = cpool.tile([P, KC, P], F16, name="wq_sb")
        nc.scalar.dma_start(wq_sb, wq.rearrange("(kc p) m -> p kc m", p=P))
        wk_sb = cpool.tile([P, KC, P], F16, name="wk_sb")
        nc.scalar.dma_start(wk_sb, wk.rearrange("(kc p) m -> p kc m", p=P))
        wqr_sb = cpool.tile([P, KC, P], F16, name="wqr_sb")
        nc.scalar.dma_start(wqr_sb, wqr.rearrange("(kc p) m -> p kc m", p=P))
        wv_sb = cpool.tile([P, KC, P], F16, name="wv_sb")
        nc.scalar.dma_start(wv_sb, wv.rearrange("(kc p) m -> p kc m", p=P))
        wkt_sb = cpool.tile([2 * DH, DH], F16, name="wkt_sb")
        nc.scalar.dma_start(wkt_sb[0:DH, :], wkt[:, :])
        nc.scalar.dma_start(wkt_sb[DH : 2 * DH, :], wkt[:, :])
        wo_sb = cpool.tile([P, DIM], F16, name="wo_sb")
        nc.scalar.dma_start(wo_sb, wo[:, :])

        # ---- x^T pre-transposed + pre-cast by host; both HWDGE queues ----
        xT = xp.tile([P, KC, N], F16, tag="x", name="xT")
        for kc in range(KC):
            eng = nc.sync if kc % 2 == 0 else nc.scalar
            eng.dma_start(xT[:, kc, :], xt[kc * P : (kc + 1) * P, :])

        # ---- persistent SBUF tiles ----
        sqT = qp.tile([P, N], F16, tag="q", name="sqT")
        skT = sb.tile([P, N], F16, name="skT")
        rqT = sb.tile([P, N], F16, name="rqT")
        rv16 = sb.tile([P, NT, P], F16, name="rv16")
        ET0 = sb.tile([P, NT, N], F16, name="ET0")
        ET1 = sb.tile([P, NT - 4, N], F16, name="ET1")
        uT16 = [sb.tile([P, N], F16, name=f"uT16_{h}") for h in range(HPC)]
        un = [sb.tile([P, NT, R, DH], F16, name=f"un{h}") for h in range(HPC)]
        rqw = [sb.tile([P, NT, DH], F16, name=f"rqw{h}") for h in range(HPC)]
        uc2 = sb.tile([P, NT, P], F16, name="uc2")
        state = {"ET1d": None, "uf2": None}

        def et(h, jt):
            if h == 0:
                return ET0, jt
            if jt < NT - 4:
                return ET1, jt
            return state["ET1d"], jt - (NT - 4)

        # ---- projections: q, k (through the st tag; ACT copies) ----
        def proj(wsb, dst):
            for icp in range(2):
                pp = pst.tile([P, 1024], F32, tag="st", name="pp")
                for half in range(2):
                    i0 = icp * 1024 + half * 512
                    for kc in range(KC):
                        nc.tensor.matmul(
                            pp[:, half * 512 : (half + 1) * 512],
                            wsb[:, kc, :],
                            xT[:, kc, i0 : i0 + 512],
                            start=(kc == 0),
                            stop=(kc == KC - 1),
                            skip_group_check=True,
                        )
                nc.scalar.copy(
                    out=dst[:, icp * 1024 : (icp + 1) * 1024], in_=pp
                )

        proj(wq_sb, sqT)
        proj(wk_sb, skT)

        def rv_chunk(ic):
            pv = pwk.tile([P, 512], F32, tag="wk", name="pv")
            for kc in range(KC):
                nc.tensor.matmul(
                    pv,
                    wv_sb[:, kc, :],
                    xT[:, kc, ic * 512 : (ic + 1) * 512],
                    start=(kc == 0),
                    stop=(kc == KC - 1),
                )
            rvT_c = hd.tile([P, 512], F16, tag="rvT", name="rvT_c")
            nc.vector.tensor_copy(out=rvT_c, in_=pv)
            for t in range(4):
                jt = ic * 4 + t
                pt = pwk.tile([P, P], F16, tag="wk", name="pt")
                nc.tensor.transpose(pt, rvT_c[:, t * P : (t + 1) * P], ident)
                nc.vector.tensor_copy(out=rv16[:, jt, :], in_=pt)

        def rq_chunk(icp):
            for half in range(2):
                i0 = icp * 1024 + half * 512
                pp = pwk.tile([P, 512], F32, tag="wk", name="prq")
                for kc in range(KC):
                    nc.tensor.matmul(
                        pp,
                        wqr_sb[:, kc, :],
                        xT[:, kc, i0 : i0 + 512],
                        start=(kc == 0),
                        stop=(kc == KC - 1),
                    )
                nc.vector.tensor_copy(out=rqT[:, i0 : i0 + 512], in_=pp)

        rv_chunk(0)  # av(h0, jt=0) needs rv16[0..3] early

        # -------- trailing work emitter (quarter-pipelined) ------
        def trail_gen(h):
            last = h == HPC - 1
            hs = slice(h * DH, (h + 1) * DH)
            cp_eng = nc.scalar if last else nc.vector
            # AV chunks c=2,3
            for c in (2, 3):
                pu = pwk.tile([P, 512], F32, tag="wk", name=f"pu{h}{c}")
                for jt in range(NT):
                    def mm(jt=jt, c=c, pu=pu):
                        t_, lj = et(h, jt)
                        nc.tensor.matmul(
                            pu,
                            rv16[:, jt, :],
                            t_[:, lj, c * 512 : (c + 1) * 512],
                            start=(jt == 0),
                            stop=(jt == NT - 1),
                            skip_group_check=True,
                        )
                    yield mm
                def cp(c=c, pu=pu):
                    nc.vector.tensor_copy(
                        out=uT16[h][:, c * 512 : (c + 1) * 512], in_=pu
                    )
                yield cp
            # per-quarter: tree, Z, unT, rqw, stage2 (and for the last head:
            # ucT + out-projection)
            for q in range(4):
                cs = slice(q * 512, (q + 1) * 512)

                def tree(q=q, cs=cs):
                    for g in range(4):
                        ta, a = et(h, 4 * g)
                        tb, b_ = et(h, 4 * g + 1)
                        tc_, c_ = et(h, 4 * g + 2)
                        td, d_ = et(h, 4 * g + 3)
                        nc.vector.tensor_tensor(
                            ta[:, a, cs], ta[:, a, cs], tb[:, b_, cs], ALU.add
                        )
                        nc.vector.tensor_tensor(
                            tc_[:, c_, cs], tc_[:, c_, cs], td[:, d_, cs],
                            ALU.add,
                        )
                        nc.vector.tensor_tensor(
                            ta[:, a, cs], ta[:, a, cs], tc_[:, c_, cs], ALU.add
                        )
                yield tree
                pz = pwk.tile([1, 512], F32, tag="wk", name=f"pz{h}{q}")
                for g in range(4):
                    def mm(q=q, g=g, pz=pz, cs=cs):
                        t_, lj = et(h, 4 * g)
                        nc.tensor.matmul(
                            pz,
                            ones16,
                            t_[:, lj, cs],
                            start=(g == 0),
                            stop=(g == 3),
                        )
                    yield mm
                def zcp(q=q, pz=pz, cs=cs):
                    zrow_c = hd.tile(
                        [1, 512], F32, tag="zrow", name="zrow_c"
                    )
                    nc.vector.tensor_copy(out=zrow_c, in_=pz)
                    nc.sync.dma_start(zscr[h : h + 1, cs], zrow_c[:, :])
                yield zcp
                for it in range(4 * q, 4 * q + 4):
                    def tr(it=it):
                        pt2 = pwk.tile([P, P], F16, tag="wk", name="pt2")
                        nc.tensor.transpose(
                            pt2, uT16[h][:, it * P : (it + 1) * P], ident
                        )
                        if last:
                            cp_eng.copy(out=un[h][:, it], in_=pt2)
                        else:
                            cp_eng.tensor_copy(out=un[h][:, it], in_=pt2)
                    yield tr
                    def qm(it=it):
                        pq = pwk.tile([P, DH], F32, tag="wk", name="pq")
                        nc.tensor.matmul(
                            pq,
                            rqT[hs, it * P : (it + 1) * P],
                            wkt_sb[hs, :],
                            start=True,
                            stop=True,
                        )
                        nc.vector.tensor_copy(out=rqw[h][:, it], in_=pq)
                    yield qm

                def s2(q=q):
                    stage2_q(h, q)
                yield s2
                if last:
                    # both heads' uc2 halves for this quarter are now done
                    for it in range(4 * q, 4 * q + 4):
                        def out_it(it=it):
                            pf = pwk.tile([P, P], F16, tag="wk", name="pf")
                            nc.tensor.transpose(pf, uc2[:, it], ident)
                            uf2 = state["uf2"]
                            nc.scalar.copy(
                                out=uf2[:, it * P : (it + 1) * P], in_=pf
                            )
                            py = pst.tile([P, DIM], F32, tag="st", name="py")
                            nc.tensor.matmul(
                                py,
                                uf2[:, it * P : (it + 1) * P],
                                wo_sb,
                                start=True,
                                stop=True,
                            )
                            ysb = hd.tile([P, DIM], F16, tag="ysb", name="ysb")
                            nc.scalar.copy(out=ysb, in_=py)
                            nc.sync.dma_start(y[it * P : (it + 1) * P, :], ysb)
                        yield out_it

        def stage2_q(h, q):
            """Gating for i-quarter q of head h -> uc2[:, 4q:4q+4, h*64:...]."""
            its = slice(4 * q, 4 * q + 4)
            zcol = hd.tile([P, 4], F32, tag="zcol", name="zcol")
            nc.sync.dma_start(
                zcol, zscr.rearrange("b (it p) -> b p it", p=P)[h][:, its]
            )
            sims = []
            for r in range(R):
                prod = hd.tile([P, 4, DH], F16, tag="prod", name="prod")
                nc.vector.tensor_tensor(
                    prod, un[h][:, its, r, :], rqw[h][:, its, :], ALU.mult
                )
                s_ = hd.tile([P, 4], F32, tag=f"sims{r}", name=f"sims{r}")
                nc.vector.tensor_reduce(
                    s_, prod, axis=mybir.AxisListType.X, op=ALU.add
                )
                sims.append(s_)
            zinv = hd.tile([P, 4], F32, tag="zinv", name="zinv")
            nc.vector.reciprocal(zinv, zcol)
            gd = hd.tile([P, 4], F32, tag="gd", name="gd")
            nc.vector.tensor_tensor(gd, sims[0], sims[1], ALU.subtract)
            nc.vector.tensor_tensor(gd, gd, zinv, ALU.mult)
            p0 = hd.tile([P, 4], F32, tag="p0", name="p0")
            nc.scalar.activation(p0, gd, AF.Exp)
            w_ = hd.tile([P, 4], F32, tag="w_", name="w_")
            nc.vector.tensor_scalar_add(w_, p0, 1.0)
            nc.vector.reciprocal(w_, w_)
            nc.vector.tensor_tensor(w_, w_, zinv, ALU.mult)
            a0z = hd.tile([P, 4], F32, tag="a0z", name="a0z")
            nc.vector.tensor_tensor(a0z, w_, p0, ALU.mult)
            ucs = uc2[:, its, h * DH : (h + 1) * DH]
            nc.vector.tensor_tensor(
                ucs, un[h][:, its, 0, :],
                a0z[:, :, None].to_broadcast((P, 4, DH)), ALU.mult,
            )
            t1 = hd.tile([P, 4, DH], F16, tag="t1", name="t1")
            nc.vector.tensor_tensor(
                t1, un[h][:, its, 1, :],
                w_[:, :, None].to_broadcast((P, 4, DH)), ALU.mult,
            )
            nc.vector.tensor_tensor(ucs, ucs, t1, ALU.add)

        # -------- main per-head jt-loops --------
        trail = None
        for h in range(HPC):
            hs = slice(h * DH, (h + 1) * DH)
            if h == 1:
                trail = trail_gen(0)
                state["ET1d"] = xp.tile([P, 4, N], F16, tag="x", name="ET1d")
                state["uf2"] = qp.tile([P, N], F16, tag="q", name="uf2")
            av01 = pav.tile([P, 1024], F32, tag="av", name=f"av01_{h}")
            for jt in range(NT):
                th, lj = et(h, jt)
                for icp in range(2):
                    st = pst.tile([P, 1024], F32, tag="st", name="st")
                    for half in range(2):
                        i0 = icp * 1024 + half * 512
                        nc.tensor.matmul(
                            st[:, half * 512 : (half + 1) * 512],
                            skT[hs, jt * P : (jt + 1) * P],
                            sqT[hs, i0 : i0 + 512],
                            start=True,
                            stop=True,
                            skip_group_check=True,
                        )
                    nc.scalar.activation(
                        th[:, lj, icp * 1024 : (icp + 1) * 1024], st, AF.Exp
                    )
                if jt >= 2:
                    ajt = jt - 2
                    ta, la = et(h, ajt)
                    for c in range(2):
                        nc.tensor.matmul(
                            av01[:, c * 512 : (c + 1) * 512],
                            rv16[:, ajt, :],
                            ta[:, la, c * 512 : (c + 1) * 512],
                            start=(ajt == 0),
                            stop=False,
                            skip_group_check=True,
                        )
                if h == 0:
                    if jt in (1, 3, 5):
                        rv_chunk((jt + 1) // 2)
                    elif jt in (7, 9):
                        rq_chunk((jt - 7) // 2)
                else:
                    for _ in range(6):
                        op = next(trail, None)
                        if op is None:
                            break
                        op()
            for ajt in (NT - 2, NT - 1):
                ta, la = et(h, ajt)
                for c in range(2):
                    nc.tensor.matmul(
                        av01[:, c * 512 : (c + 1) * 512],
                        rv16[:, ajt, :],
                        ta[:, la, c * 512 : (c + 1) * 512],
                        start=False,
                        stop=(ajt == NT - 1),
                        skip_group_check=True,
                    )
            nc.vector.tensor_copy(out=uT16[h][:, 0:1024], in_=av01)
            if h == 1:
                for op in trail:
                    op()

        for op in trail_gen(HPC - 1):
            op()


def build_program():
    nc = bacc.Bacc(None, target_bir_lowering=False)
    xt = nc.declare_dram_parameter("xt", [DIM, N], F16, isOutput=False)
    wq = nc.declare_dram_parameter("wq", [DIM, P], F16, isOutput=False)
    wk = nc.declare_dram_parameter("wk", [DIM, P], F16, isOutput=False)
    wv = nc.declare_dram_parameter("wv", [DIM, P], F16, isOutput=False)
    wqr = nc.declare_dram_parameter("wqr", [DIM, P], F16, isOutput=False)
    wkt = nc.declare_dram_parameter("wkt", [DH, DH], F16, isOutput=False)
    wo = nc.declare_dram_parameter("wo", [P, DIM], F16, isOutput=False)
    y = nc.declare_dram_parameter("y", [N, DIM], F16, isOutput=True)
    zscr = nc.dram_tensor("zscr", [HPC, N], F32)

    with tile.TileContext(nc) as tc:
        _emit(tc, xt, wq, wk, wv, wqr, wkt, wo, y, zscr)
    nc.compile()
    return nc


_NC_CACHE = None


def _get_program():
    global _NC_CACHE
    if _NC_CACHE is None:
        _NC_CACHE = build_program()
    return _NC_CACHE


def make_in_maps(inputs):
    x = np.asarray(inputs["x"], dtype=np.float32)
    wq_s = np.asarray(inputs["wq_s"], dtype=np.float32)
    wk_s = np.asarray(inputs["wk_s"], dtype=np.float32)
    wv_r = np.asarray(inputs["wv_r"], dtype=np.float32)
    wq_r = np.asarray(inputs["wq_r"], dtype=np.float32)
    wk_ret = np.asarray(inputs["wk_ret"], dtype=np.float32)
    w_out = np.asarray(inputs["w_out"], dtype=np.float32)
    scale = np.float32(DH**-0.5)

    f16 = np.float16
    in_maps = []
    for c in range(NCORES):
        b, hp = divmod(c, NCORES // B)
        cols = slice(hp * P, (hp + 1) * P)
        in_maps.append(
            {
                "xt": np.ascontiguousarray(x[b].T).astype(f16),
                "wq": (np.ascontiguousarray(wq_s[:, cols]) * scale).astype(f16),
                "wk": np.ascontiguousarray(wk_s[:, cols]).astype(f16),
                "wv": wv_r.astype(f16),
                "wqr": (np.ascontiguousarray(wq_r[:, cols]) * scale).astype(f16),
                "wkt": np.ascontiguousarray(wk_ret.T).astype(f16),
                "wo": np.ascontiguousarray(w_out[hp * P : (hp + 1) * P, :]).astype(f16),
            }
        )
    return in_maps


def run(inputs, trace=False, **kw):
    res = run_bass_kernel_spmd(
        _get_program(), make_in_maps(inputs), list(range(NCORES)), trace=trace, **kw
    )
    out = np.zeros((B, N, DIM), np.float32)
    for c in range(NCORES):
        out[c // (NCORES // B)] += np.asarray(res.results[c]["y"], np.float32)
    return out, res


def kernel(**inputs):
    out, _ = run(inputs)
    return out
